# revision 1
# baseline (speedup 1.0000x reference)
"""Multi-head self-attention (B=4, S=1024, D=1024, H=16, RoPE, causal) on 8
Trainium2 NeuronCores.

Sharding: 8 cores = 4 batches x 2 head-groups (8 heads each). Each core
computes QKV projections for its batch/head-group, RoPE, causal attention,
and a partial output projection (contraction over its 512 attention dims).
The host sums the two partial outputs per batch (the "all-reduce") and
concatenates batches.

Device layout notes:
- Weights are passed transposed; Q/K projection output dims are permuted to
  rotate-half order (evens then odds within each head) so RoPE works on
  contiguous 32-column halves. Permuting Q and K identically leaves Q.K^T
  unchanged.
- Logits are computed transposed (L^T[k, q]) so softmax sums reduce over the
  PSUM partition axis via a ones-column appended to V, and the attention
  output arrives as attn^T[c, q] which feeds the output projection directly.
- Matmul operands are float32r (1.5 cyc/row vs 4 for fp32, ~1e-4 rel err).
- Softmax 1/sum is computed on a [8,128] reshape (DMA) and broadcast across
  partitions via a DRAM round-trip, keeping the PE out of it.
- The final output is produced transposed (y^T[o, q]); the host transposes.
"""

import numpy as np

import concourse.bass as bass
import concourse.mybir as mybir
import concourse.tile as tile
from concourse.bass import ts
from concourse.bass_utils import run_bass_kernel_spmd
from concourse.masks import make_identity, make_upper_triangular

B, S, D = 4, 1024, 1024
H = 16  # total heads
HG = 8  # heads per core (head-group)
DK = 64  # head dim
DG = HG * DK  # 512, per-core projection width
ROPE_THETA = 10000.0
P = 128  # partitions
NS = S // P  # 8 s-tiles
ND = D // P  # 8 d-chunks
F32 = mybir.dt.float32
F32R = mybir.dt.float32r

_uid = [0]


def _split_excess_waits(nc, limit=1):
    """This container's walrus rejects >1 sync waits on the kernel-tail
    Drain; move excess waits onto same-engine NoOps inserted before it."""
    for f in nc.m.functions:
        for blk in f.blocks:
            insts = list(blk.instructions)
            out = []
            changed = False
            for inst in insts:
                si = inst.sync_info
                if si is not None and si.on_wait and len(si.on_wait) > limit:
                    waits = list(si.on_wait)
                    head, tail = waits[:-limit], waits[-limit:]
                    for i in range(0, len(head), limit):
                        _uid[0] += 1
                        nop = mybir.InstNoOp(
                            name=f"waitsplit-{_uid[0]}", ins=[], outs=[]
                        )
                        nop.engine = inst.engine
                        nop.sync_info = mybir.SyncInfo(
                            on_wait=head[i : i + limit], on_update=[]
                        )
                        out.append(nop)
                    si.on_wait = tail
                    changed = True
                out.append(inst)
            if changed:
                blk.instructions = out
    return nc


def build_nc():
    nc = bass.Bass("TRN2")
    MMD = F32R  # matmul operand dtype
    xT = nc.dram_tensor("xT", [D, S], MMD, kind="ExternalInput")
    wqT = nc.dram_tensor("wqT", [D, DG], MMD, kind="ExternalInput")
    wkT = nc.dram_tensor("wkT", [D, DG], MMD, kind="ExternalInput")
    wvT = nc.dram_tensor("wvT", [D, DG], MMD, kind="ExternalInput")
    woT = nc.dram_tensor("woT", [DG, D], MMD, kind="ExternalInput")
    cos8 = nc.dram_tensor("cos8", [S, HG * 32], F32, kind="ExternalInput")
    sin8 = nc.dram_tensor("sin8", [S, HG * 32], F32, kind="ExternalInput")
    yT = nc.dram_tensor("yT", [D, S], F32, kind="ExternalOutput")
    # DRAM scratch for the softmax 1/sum partition-broadcast round-trip
    rsum = nc.dram_tensor("rsum", [HG, S], F32)
    rrec = nc.dram_tensor("rrec", [HG, S], F32)

    with tile.TileContext(nc) as tc:
        with (
            tc.tile_pool(name="const", bufs=1) as constp,
            tc.tile_pool(name="wq", bufs=1) as wqp,
            tc.tile_pool(name="big", bufs=1) as bigp,
        ):
            # constants (f32r tiles can't be memset directly; build in f32
            # and convert via ACT copy, which rounds)
            identf = constp.tile([P, P], F32, tag="identf")
            make_identity(nc, identf[:, :])
            ident = constp.tile([P, P], MMD, tag="ident")
            nc.scalar.copy(out=ident[:, :], in_=identf[:, :])
            # ztril: [zeros(384) | lower-keep causal mask(128)] — sliced per
            # k-tile to zero both the below-diagonal block and the q<k-tile
            # part of each uniform 512-wide logits chunk
            ztril = constp.tile([P, 512], F32, tag="ztril")
            nc.vector.memset(ztril[:, :], 0.0)
            make_upper_triangular(nc, ztril[:, 384:512], val=1.0, diag=True)
            onesf = constp.tile([P, HG], F32, tag="onesf")
            nc.vector.memset(onesf[:, :], 1.0)

            # resident weights, one batched tile+DMA per weight
            # (wo is loaded at the start of phase B: phase D needs it latest)
            wq_all = wqp.tile([P, ND, DG], MMD, tag="wq", name="wq_all")
            wk_all = wqp.tile([P, ND, DG], MMD, tag="wk", name="wk_all")
            wv_all = wqp.tile([P, ND, DG], MMD, tag="wv", name="wv_all")
            wo_all = wqp.tile([P, DG // P, D], MMD, tag="wo", name="wo_all")
            wq_sb = [wq_all[:, c, :] for c in range(ND)]
            wk_sb = [wk_all[:, c, :] for c in range(ND)]
            wv_sb = [wv_all[:, c, :] for c in range(ND)]
            wo_sb = [wo_all[:, c, :] for c in range(DG // P)]

            def load_weights():
                # per-chunk DMAs: the first projection only needs chunk 0,
                # so don't gate it on the full 2MB transfer
                for c in range(ND):
                    for w_all, wT in ((wq_all, wqT), (wk_all, wkT), (wv_all, wvT)):
                        nc.sync.dma_start(
                            out=w_all[:, c, :], in_=wT[ts(c, P), :]
                        )

            def load_wo():
                nc.sync.dma_start(
                    out=wo_all[:, :, :],
                    in_=woT[:, :].rearrange("(c p) o -> p c o", p=P),
                )

            # persistent activations
            qt_sb = [bigp.tile([P, S], MMD, tag=f"qt{p}", name=f"qt{p}") for p in range(4)]
            kt_sb = [bigp.tile([P, S], MMD, tag=f"kt{p}", name=f"kt{p}") for p in range(4)]
            v_sb = [bigp.tile([P, HG, DK + 1], MMD, tag=f"v{j}", name=f"v{j}") for j in range(NS)]
            at_sb = [bigp.tile([P, S], MMD, tag=f"at{p}", name=f"at{p}") for p in range(4)]

            # ---------------- Phase A: projections + RoPE + transposes ----
            with (
                tc.tile_pool(name="pa_psum", bufs=2, space="PSUM") as pap,
                tc.tile_pool(name="tp_psum", bufs=2, space="PSUM") as tpp,
                tc.tile_pool(name="pa_sbuf", bufs=3) as pas,
                tc.tile_pool(name="rope", bufs=4) as ropep,
            ):
                for i in range(NS):
                    xt_all = pas.tile([P, ND, P], MMD, tag="xt", name="xt")
                    nc.sync.dma_start(
                        out=xt_all[:, :, :],
                        in_=xT[:, :].rearrange("(c p) s -> p c s", p=P)[
                            :, :, ts(i, P)
                        ],
                    )
                    xt = [xt_all[:, c, :] for c in range(ND)]
                    cs = pas.tile([P, HG * 32], F32, tag="cos")
                    sn = pas.tile([P, HG * 32], F32, tag="sin")
                    nc.sync.dma_start(out=cs[:, :], in_=cos8[ts(i, P), :])
                    nc.sync.dma_start(out=sn[:, :], in_=sin8[ts(i, P), :])
                    if i == 0:
                        # emit weight loads after s-tile 0's x/cos/sin so the
                        # first projections aren't queued behind 8 MB of
                        # weight DMA
                        load_weights()

                    qp = pap.tile([P, DG], F32, tag="q")
                    kp = pap.tile([P, DG], F32, tag="k")
                    vp = pap.tile([P, DG], F32, tag="v")
                    for c in range(ND):
                        st = c == 0
                        sp = c == ND - 1
                        nc.tensor.matmul(
                            qp[:, :], lhsT=xt[c], rhs=wq_sb[c],
                            start=st, stop=sp,
                        )
                        nc.tensor.matmul(
                            kp[:, :], lhsT=xt[c], rhs=wk_sb[c],
                            start=st, stop=sp,
                        )
                        nc.tensor.matmul(
                            vp[:, :], lhsT=xt[c], rhs=wv_sb[c],
                            start=st, stop=sp,
                        )

                    # V -> SBUF with a ones column per head (softmax sums)
                    nc.scalar.copy(
                        out=v_sb[i][:, :, 0:DK],
                        in_=vp[:, :].rearrange("p (h c) -> p h c", h=HG),
                    )
                    nc.scalar.copy(
                        out=v_sb[i][:, :, DK : DK + 1],
                        in_=onesf[:, :].rearrange("p (h c) -> p h c", c=1),
                    )

                    # RoPE on q/k (rotate-half layout: per head [32 even|32 odd])
                    cs3 = cs[:, :].rearrange("p (h c) -> p h c", h=HG)
                    sn3 = sn[:, :].rearrange("p (h c) -> p h c", h=HG)
                    for src, dst_tag in ((qp, "qr"), (kp, "kr")):
                        sv = src[:, :].rearrange(
                            "p (h t c) -> p h t c", h=HG, t=2
                        )
                        ev, od = sv[:, :, 0, :], sv[:, :, 1, :]
                        r = pas.tile([P, DG], MMD, tag=dst_tag, name=dst_tag)
                        rv = r[:, :].rearrange("p (h t c) -> p h t c", h=HG, t=2)
                        t1 = ropep.tile([P, HG * 32], F32, tag="t1")
                        t2 = ropep.tile([P, HG * 32], F32, tag="t2")
                        t13 = t1[:, :].rearrange("p (h c) -> p h c", h=HG)
                        t23 = t2[:, :].rearrange("p (h c) -> p h c", h=HG)
                        nc.vector.tensor_mul(t13, ev, cs3)
                        nc.vector.tensor_mul(t23, od, sn3)
                        nc.vector.tensor_sub(rv[:, :, 0, :], t13, t23)
                        t3 = ropep.tile([P, HG * 32], F32, tag="t3")
                        t4 = ropep.tile([P, HG * 32], F32, tag="t4")
                        t33 = t3[:, :].rearrange("p (h c) -> p h c", h=HG)
                        t43 = t4[:, :].rearrange("p (h c) -> p h c", h=HG)
                        nc.vector.tensor_mul(t33, ev, sn3)
                        nc.vector.tensor_mul(t43, od, cs3)
                        nc.vector.tensor_add(rv[:, :, 1, :], t33, t43)

                        # transpose head-pairs into [d, s] tiles
                        dst_tiles = qt_sb if dst_tag == "qr" else kt_sb
                        for p in range(4):
                            tp = tpp.tile([P, P], MMD, tag="tp")
                            nc.tensor.transpose(
                                tp[:, :], r[:, ts(p, P)], ident[:, :]
                            )
                            nc.scalar.copy(
                                out=dst_tiles[p][:, ts(i, P)], in_=tp[:, :]
                            )

            # ---------------- Phase B: attention per head ------------------
            # Per-chunk [128,512] logits/probability tiles: 4 one-bank lg
            # slots keep QK running ahead of exp; EV is emitted ~2 k-tiles
            # behind QK. Softmax 1/sum runs per q-half to shorten the
            # exposed chain latency after the last head.
            with (
                tc.tile_pool(name="attn_psum", bufs=2, space="PSUM") as atp,
                tc.tile_pool(name="lg_psum", bufs=4, space="PSUM") as lgp,
                tc.tile_pool(name="pt_pool", bufs=7) as ptp,
                tc.tile_pool(name="sm_pool", bufs=2) as smp,
            ):
                load_wo()

                def emit_ev(h, ap, j, cpts):
                    for lo, hi, pt in cpts:
                        nc.tensor.matmul(
                            ap[:, lo:hi],
                            lhsT=v_sb[j][:, h, :],
                            rhs=pt[:, 0 : hi - lo],
                            start=(j == 0), stop=(j == NS - 1),
                            skip_group_check=True,
                        )

                for h in (1, 0, 3, 2, 5, 4, 7, 6):
                    pair, poff = h // 2, 64 * (h % 2)
                    ap = atp.tile([DK + 1, S], F32, tag="attn", name=f"ap{h}")
                    pending = []
                    for j in range(NS):
                        q0 = P * j
                        cpts = []
                        for c0 in (0, 512):
                            lo = max(q0, c0)
                            n = c0 + 512 - lo
                            if n <= 0:
                                continue
                            lg = lgp.tile([P, 512], F32, tag="lg", name="lg")
                            nc.tensor.matmul(
                                lg[:, 0:n],
                                lhsT=kt_sb[pair][poff : poff + DK, ts(j, P)],
                                rhs=qt_sb[pair][poff : poff + DK, lo : lo + n],
                                start=True, stop=True,
                            )
                            pt = ptp.tile([P, 512], MMD, tag="pt", name="pt")
                            nc.scalar.activation(
                                out=pt[:, 0:n], in_=lg[:, 0:n],
                                func=mybir.ActivationFunctionType.Exp,
                                scale=0.125,
                            )
                            if lo == q0:
                                # diagonal block sits at the chunk start
                                nc.vector.tensor_mul(
                                    pt[:, 0:P], pt[:, 0:P], ztril[:, 384:512]
                                )
                            cpts.append((lo, c0 + 512, pt))
                        pending.append((j, cpts))
                        if len(pending) > 2:
                            emit_ev(h, ap, *pending.pop(0))
                    for args in pending:
                        emit_ev(h, ap, *args)

                    # softmax normalization per q-half: copy sums to SBUF,
                    # [4,128] reshape via DRAM, reciprocal, partition-
                    # broadcast DMA read back, multiply.
                    for hx in (0, 512):
                        sr = smp.tile([P, 512], F32, tag="sr", name="sr")
                        nc.scalar.copy(
                            out=sr[64:65, :], in_=ap[DK : DK + 1, hx : hx + 512]
                        )
                        nc.sync.dma_start(
                            out=rsum[h, hx : hx + 512].rearrange(
                                "(o c) -> o c", o=1
                            ),
                            in_=sr[64:65, :],
                        )
                        rs8 = smp.tile([4, P], F32, tag="rs8", name="rs8")
                        nc.sync.dma_start(
                            out=rs8[:, :],
                            in_=rsum[h, hx : hx + 512].rearrange(
                                "(r c) -> r c", r=4
                            ),
                        )
                        rc8 = smp.tile([4, P], F32, tag="rc8", name="rc8")
                        nc.vector.reciprocal(out=rc8[:, :], in_=rs8[:, :])
                        nc.sync.dma_start(
                            out=rrec[h, hx : hx + 512].rearrange(
                                "(r c) -> r c", r=4
                            ),
                            in_=rc8[:, :],
                        )
                        row = rrec[h, hx : hx + 512]
                        bc_src = bass.AP(
                            tensor=row.tensor,
                            offset=row.offset,
                            ap=[[0, DK], [1, 512]],
                        )
                        bcs = smp.tile([DK, 512], F32, tag="bcs", name="bcs")
                        nc.sync.dma_start(out=bcs[:, :], in_=bc_src)
                        if poff == 0:
                            nc.vector.tensor_mul(
                                at_sb[pair][0:DK, hx : hx + 512],
                                ap[0:DK, hx : hx + 512],
                                bcs[:, :],
                            )
                        else:
                            tmp = smp.tile([DK, 512], MMD, tag="odd", name="odd")
                            nc.vector.tensor_mul(
                                tmp[:, :], ap[0:DK, hx : hx + 512], bcs[:, :]
                            )
                            nc.sync.dma_start(
                                out=at_sb[pair][DK:P, hx : hx + 512],
                                in_=tmp[:, :],
                            )

                # ---------- output projection (shares lg psum slots so it
                # can start while the last head's softmax chain drains) ----
                with tc.tile_pool(name="y_sbuf", bufs=4) as ys:
                    for og in range(0, ND, 2):
                        group = range(og, min(og + 2, ND))
                        ypts = {
                            (o, qc): lgp.tile(
                                [P, 512], F32, tag="lg", name=f"y{o}_{qc}"
                            )
                            for o in group
                            for qc in (0, 512)
                        }
                        for c in range(DG // P):
                            for o in group:
                                for qc in (0, 512):
                                    nc.tensor.matmul(
                                        ypts[o, qc][:, :],
                                        lhsT=wo_sb[c][:, ts(o, P)],
                                        rhs=at_sb[c][:, qc : qc + 512],
                                        start=(c == 0), stop=(c == DG // P - 1),
                                    )
                        for o in group:
                            ysb = ys.tile([P, S], F32, tag="ysb", name="ysb")
                            nc.scalar.copy(
                                out=ysb[:, 0:512], in_=ypts[o, 0][:, :]
                            )
                            nc.scalar.copy(
                                out=ysb[:, 512:1024], in_=ypts[o, 512][:, :]
                            )
                            nc.sync.dma_start(out=yT[ts(o, P), :], in_=ysb[:, :])

    _split_excess_waits(nc)
    return nc


_NC_CACHE = {}


def _get_nc():
    if "nc" not in _NC_CACHE:
        _NC_CACHE["nc"] = build_nc()
    return _NC_CACHE["nc"]


# rotate-half permutation within each head: evens then odds
_PERM = np.concatenate([np.arange(0, DK, 2), np.arange(1, DK, 2)])


def _host_prep(x, Wq, Wk, Wv, Wo, token_positions):
    """Build the 8 per-core input dicts."""
    inv_freq = 1.0 / (ROPE_THETA ** (np.arange(0, DK, 2, dtype=np.float32) / DK))
    in_maps = []
    for core in range(8):
        b, g = core // 2, core % 2
        heads = np.arange(HG * g, HG * (g + 1))
        rows_qk = (heads[:, None] * DK + _PERM[None, :]).reshape(-1)
        rows_v = (heads[:, None] * DK + np.arange(DK)[None, :]).reshape(-1)
        pos = token_positions[b].astype(np.float32)  # [S]
        ang = pos[:, None] * inv_freq[None, :]  # [S, 32]
        cos8 = np.tile(np.cos(ang), (1, HG)).astype(np.float32)
        sin8 = np.tile(np.sin(ang), (1, HG)).astype(np.float32)
        in_maps.append(
            {
                "xT": np.ascontiguousarray(x[b].T),
                "wqT": np.ascontiguousarray(Wq[rows_qk, :].T),
                "wkT": np.ascontiguousarray(Wk[rows_qk, :].T),
                "wvT": np.ascontiguousarray(Wv[rows_v, :].T),
                "woT": np.ascontiguousarray(Wo[:, rows_v].T),
                "cos8": cos8,
                "sin8": sin8,
            }
        )
    return in_maps


def kernel(x, Wq, Wk, Wv, Wo, token_positions, _trace=False):
    x = np.asarray(x, dtype=np.float32)
    Wq = np.asarray(Wq, dtype=np.float32)
    Wk = np.asarray(Wk, dtype=np.float32)
    Wv = np.asarray(Wv, dtype=np.float32)
    Wo = np.asarray(Wo, dtype=np.float32)
    token_positions = np.asarray(token_positions)

    nc = _get_nc()
    in_maps = _host_prep(x, Wq, Wk, Wv, Wo, token_positions)
    res = run_bass_kernel_spmd(nc, in_maps, core_ids=list(range(8)), trace=_trace)
    if _trace:
        kernel.last_exec_time_ns = res.exec_time_ns
        kernel.last_results = res

    y = np.empty((B, S, D), dtype=np.float32)
    for b in range(B):
        yT0 = res.results[2 * b]["yT"]
        yT1 = res.results[2 * b + 1]["yT"]
        y[b] = (yT0 + yT1).T
    return y



# revision 15
# speedup vs baseline: 1.2756x; 1.2756x over previous
"""Multi-head self-attention (B=4, S=1024, D=1024, H=16, RoPE, causal) on 8
Trainium2 NeuronCores.

Sharding: 8 cores = 4 batches x 2 head-groups (8 heads each). Each core
computes QKV projections for its batch/head-group, RoPE, causal attention,
and a partial output projection (contraction over its 512 attention dims).
The host sums the two partial outputs per batch (the "all-reduce") and
concatenates batches.

v2 design (vs the f32r baseline):
- All matmul operands are bf16 (1 cyc/row at any tile size; half the DMA).
- Q/K are projected TRANSPOSED (weights stationary, x streamed) so no PE
  transposes are needed; RoPE runs in [dg, s] layout using a partition
  block-swap done by 4 small SBUF->SBUF DMAs plus 3 elementwise ops
  (2 on DVE, the add on GpSimd).
- Causal attention is split into two q-halves. Half-0 (q in [0,512), k-tiles
  0..3) only needs the first half of phase A, so its 8 heads interleave with
  the second half of the projections, overlapping ACT exp time with PE time.
- Logits are computed transposed (L^T[k, q]); softmax sums come from a ones
  column appended to V; exp instructions are merged across j k-tiles
  (contiguous packed columns) to amortize ACT overhead.
- Softmax 1/sum: DVE reciprocal_approx_fast on the [1,512] sum row, then a
  K=1 PE outer-product broadcasts it across 64 partitions (no DRAM round
  trip), then one DVE multiply produces bf16 normalized attention.
- The causal tril mask multiply runs on GpSimd (otherwise idle).
- Output projection per q-half interleaves with the other half's attention.
"""

import numpy as np

import concourse.bass as bass
import concourse.mybir as mybir
import concourse.tile as tile
from concourse.bass import ts
from concourse.bass_utils import run_bass_kernel_spmd
from concourse.masks import make_upper_triangular

B, S, D = 4, 1024, 1024
H = 16  # total heads
HG = 8  # heads per core (head-group)
DK = 64  # head dim
DG = HG * DK  # 512, per-core projection width
ROPE_THETA = 10000.0
P = 128  # partitions
NS = S // P  # 8 s-tiles
ND = D // P  # 8 d-chunks
F32 = mybir.dt.float32
F32R = mybir.dt.float32r
BF16 = mybir.dt.bfloat16
EXP = mybir.ActivationFunctionType.Exp

_uid = [0]


def _split_excess_waits(nc, limit=1):
    """This container's walrus rejects >1 sync waits on the kernel-tail
    Drain; move excess waits onto same-engine NoOps inserted before it."""
    for f in nc.m.functions:
        for blk in f.blocks:
            insts = list(blk.instructions)
            out = []
            changed = False
            for inst in insts:
                si = inst.sync_info
                if si is not None and si.on_wait and len(si.on_wait) > limit:
                    waits = list(si.on_wait)
                    head, tail = waits[:-limit], waits[-limit:]
                    for i in range(0, len(head), limit):
                        _uid[0] += 1
                        nop = mybir.InstNoOp(
                            name=f"waitsplit-{_uid[0]}", ins=[], outs=[]
                        )
                        nop.engine = inst.engine
                        nop.sync_info = mybir.SyncInfo(
                            on_wait=head[i : i + limit], on_update=[]
                        )
                        out.append(nop)
                    si.on_wait = tail
                    changed = True
                out.append(inst)
            if changed:
                blk.instructions = out
    return nc


def build_nc():
    nc = bass.Bass("TRN2")
    xT = nc.dram_tensor("xT", [D, S], BF16, kind="ExternalInput")
    wqT = nc.dram_tensor("wqT", [D, DG], BF16, kind="ExternalInput")
    wkT = nc.dram_tensor("wkT", [D, DG], BF16, kind="ExternalInput")
    wvT = nc.dram_tensor("wvT", [D, DG], BF16, kind="ExternalInput")
    woT = nc.dram_tensor("woT", [DG, D], BF16, kind="ExternalInput")
    cosT = nc.dram_tensor("cosT", [P, S], BF16, kind="ExternalInput")
    sinTs = nc.dram_tensor("sinTs", [P, S], BF16, kind="ExternalInput")
    yT = nc.dram_tensor("yT", [D, S], F32, kind="ExternalOutput")
    # DRAM scratch rows for the softmax 1/sum reshape + partition-broadcast
    rsum = nc.dram_tensor("rsum", [H, 512], F32)
    rbcd = nc.dram_tensor("rbcd", [H, 512], F32)

    with tile.TileContext(nc) as tc:
        with (
            tc.tile_pool(name="const", bufs=1) as constp,
            tc.tile_pool(name="wts", bufs=1) as wp,
            tc.tile_pool(name="big", bufs=1) as bigp,
            tc.tile_pool(name="qsw", bufs=3) as qswp,
            tc.tile_pool(name="rr", bufs=2) as rrp,
            tc.tile_pool(name="odd", bufs=2) as oddp,
            tc.tile_pool(name="ysb", bufs=3) as ysp,
        ):
            # ---- constants ----
            ztrilf = constp.tile([P, P], F32, tag="ztrilf")
            nc.vector.memset(ztrilf[:, :], 0.0)
            make_upper_triangular(nc, ztrilf[:, :], val=1.0, diag=True)
            ztril = constp.tile([P, P], BF16, tag="ztril")
            nc.vector.tensor_copy(ztril[:, :], ztrilf[:, :])

            # ---- resident weights/activations ----
            xs = wp.tile([P, ND, S], BF16, tag="xs", name="xs")
            wq_all = wp.tile([P, ND, DG], BF16, tag="wq", name="wq")
            wk_all = wp.tile([P, ND, DG], BF16, tag="wk", name="wk")
            wv_all = wp.tile([P, ND, DG], BF16, tag="wv", name="wv")
            wo_all = wp.tile([P, DG // P, D], BF16, tag="wo", name="wo")
            cs = wp.tile([P, S], BF16, tag="cs", name="cs")
            sn = wp.tile([P, S], BF16, tag="sn", name="sn")

            # xs per-chunk on sync queue (first V matmuls unblock early);
            # weights on the ACT hwdge queue run concurrently.
            for c in range(ND):
                nc.sync.dma_start(
                    out=xs[:, c, :], in_=xT[ts(c, P), :]
                )
            nc.scalar.dma_start(
                out=wv_all[:, :, :],
                in_=wvT[:, :].rearrange("(c p) o -> p c o", p=P),
            )
            nc.scalar.dma_start(out=cs[:, :], in_=cosT[:, :])
            nc.scalar.dma_start(out=sn[:, :], in_=sinTs[:, :])
            nc.scalar.dma_start(
                out=wq_all[:, :, :],
                in_=wqT[:, :].rearrange("(c p) o -> p c o", p=P),
            )
            nc.sync.dma_start(
                out=wk_all[:, :, :],
                in_=wkT[:, :].rearrange("(c p) o -> p c o", p=P),
            )
            nc.scalar.dma_start(
                out=wo_all[:, :, :],
                in_=woT[:, :].rearrange("(c p) o -> p c o", p=P),
            )

            # persistent: q^T/k^T pair tiles [128 dims, S], v tiles, at tiles
            qt_sb = [bigp.tile([P, S], BF16, tag=f"qt{p}", name=f"qt{p}") for p in range(4)]
            kt_sb = [bigp.tile([P, S], BF16, tag=f"kt{p}", name=f"kt{p}") for p in range(4)]
            v_sb = [bigp.tile([P, HG, DK + 1], BF16, tag=f"v{j}", name=f"v{j}") for j in range(NS)]
            at_sb = [bigp.tile([P, S], BF16, tag=f"at{p}", name=f"at{p}") for p in range(4)]
            for j in range(NS):
                nc.vector.memset(v_sb[j][:, :, DK : DK + 1], 1.0)

            with (
                tc.tile_pool(name="projv", bufs=3, space="PSUM") as pvp,
                tc.tile_pool(name="lg0", bufs=3, space="PSUM") as lg0p,
                tc.tile_pool(name="ap0", bufs=2, space="PSUM") as ap0p,
                tc.tile_pool(name="pt0", bufs=5) as pt0p,
            ):
                def emit_v(i):
                    vp = pvp.tile([P, DG], F32, tag="pv", name=f"v{i}")
                    for c in range(ND):
                        nc.tensor.matmul(
                            vp[:, :], lhsT=xs[:, c, ts(i, P)], rhs=wv_all[:, c, :],
                            start=(c == 0), stop=(c == ND - 1),
                        )
                    nc.scalar.copy(
                        out=v_sb[i][:, :, 0:DK],
                        in_=vp[:, :].rearrange("p (h c) -> p h c", h=HG),
                    )

                def emit_qkt(src, pair, hx):
                    # transposed projection of q or k pair tile, cols [hx, hx+512)
                    w_all = wq_all if src == "q" else wk_all
                    dst = qt_sb[pair] if src == "q" else kt_sb[pair]
                    pp = pvp.tile([P, DG], F32, tag="pv", name=f"{src}{pair}_{hx}")
                    for c in range(ND):
                        nc.tensor.matmul(
                            pp[:, :], lhsT=w_all[:, c, ts(pair, P)],
                            rhs=xs[:, c, hx : hx + 512],
                            start=(c == 0), stop=(c == ND - 1),
                        )
                    # rope: dst = pp*cos + blockswap(pp)*sin_signed
                    qs = qswp.tile([P, 512], BF16, tag="qs", name="qs")
                    nc.vector.tensor_copy(qs[:, :], pp[:, :])
                    qw = qswp.tile([P, 512], BF16, tag="qw", name="qw")
                    for g in range(4):
                        so = 32 * (g ^ 1)
                        nc.sync.dma_start(
                            out=qw[ts(g, 32), :], in_=qs[so : so + 32, :]
                        )
                    t1 = qswp.tile([P, 512], BF16, tag="t1", name="t1")
                    nc.vector.tensor_mul(t1[:, :], qs[:, :], cs[:, hx : hx + 512])
                    t2 = qswp.tile([P, 512], BF16, tag="t2", name="t2")
                    nc.vector.tensor_mul(t2[:, :], qw[:, :], sn[:, hx : hx + 512])
                    nc.gpsimd.tensor_add(dst[:, hx : hx + 512], t1[:, :], t2[:, :])

                def normalize(h, ap, hx):
                    # softmax 1/sum: copy s-row to SBUF, reshape to [128,4]
                    # (fast reciprocal shape) via SBUF DMA, reciprocal, DRAM
                    # round trip to broadcast across 64 partitions, one DVE
                    # multiply -> bf16 at tile
                    pair, poff = h // 2, 64 * (h % 2)
                    sr = rrp.tile([1, 512], F32, tag="rr", name="sr")
                    if hx == 0:
                        nc.scalar.copy(out=sr[:, :], in_=ap[DK : DK + 1, 0:512])
                    else:
                        nc.vector.tensor_copy(sr[:, :], ap[DK : DK + 1, 0:512])
                    slot = h + (HG if hx else 0)
                    nc.sync.dma_start(
                        out=rsum[slot, :].rearrange("(o c) -> o c", o=1),
                        in_=sr[:, :],
                    )
                    rs = rrp.tile([P, 4], F32, tag="rs", name="rs")
                    nc.sync.dma_start(
                        out=rs[:, :],
                        in_=rsum[slot, :].rearrange("(p c) -> p c", c=4),
                    )
                    rc = rrp.tile([P, 4], F32, tag="rc", name="rc")
                    nc.vector.reciprocal(out=rc[:, :], in_=rs[:, :])
                    nc.sync.dma_start(
                        out=rbcd[slot, :].rearrange("(p c) -> p c", c=4),
                        in_=rc[:, :],
                    )
                    row = rbcd[slot, :]
                    bc_src = bass.AP(
                        tensor=row.tensor,
                        offset=row.offset,
                        ap=[[0, DK], [1, 512]],
                    )
                    rbc = rrp.tile([DK, 512], F32, tag="rbc", name="rbc")
                    nc.sync.dma_start(out=rbc[:, :], in_=bc_src)
                    if poff == 0:
                        nc.vector.tensor_mul(
                            at_sb[pair][0:DK, hx : hx + 512],
                            ap[0:DK, 0:512], rbc[:, :],
                        )
                    else:
                        tmp = oddp.tile([DK, 512], BF16, tag="odd", name="odd")
                        nc.vector.tensor_mul(
                            tmp[:, :], ap[0:DK, 0:512], rbc[:, :]
                        )
                        nc.sync.dma_start(
                            out=at_sb[pair][DK:P, hx : hx + 512], in_=tmp[:, :]
                        )

                def emit_h0_head(h):
                    # half-0: q in [0,512), k-tiles 0..3
                    pair, poff = h // 2, 64 * (h % 2)
                    ap = ap0p.tile([P, 512], F32, tag="ap", name=f"ap0_{h}")
                    pend = []
                    for j in range(4):
                        q0 = 128 * j
                        n = 512 - q0
                        lg = lg0p.tile([P, 512], F32, tag="lg", name="lg0")
                        nc.tensor.matmul(
                            lg[:, 0:n],
                            lhsT=kt_sb[pair][poff : poff + DK, ts(j, P)],
                            rhs=qt_sb[pair][poff : poff + DK, q0:512],
                            start=True, stop=True,
                        )
                        pt = pt0p.tile([P, 512], BF16, tag="pt", name="pt0")
                        nc.scalar.activation(
                            out=pt[:, 0:n], in_=lg[:, 0:n], func=EXP, scale=0.125
                        )
                        nc.gpsimd.tensor_mul(
                            pt[:, 0:P], pt[:, 0:P], ztril[:, :]
                        )
                        pend.append((j, q0, n, pt))
                        while len(pend) > 2:
                            ev0(h, ap, *pend.pop(0))
                    for args in pend:
                        ev0(h, ap, *args)
                    normalize(h, ap, 0)

                def ev0(h, ap, j, q0, n, pt):
                    nc.tensor.matmul(
                        ap[0 : DK + 1, q0:512],
                        lhsT=v_sb[j][:, h, :], rhs=pt[:, 0:n],
                        start=(j == 0), stop=(j == 3),
                        skip_group_check=True,
                    )

                # ---- emission: phase A half-0, half-0 attn, phase A half-1
                for i in range(4):
                    emit_v(i)
                H0 = (1, 0, 3, 2, 5, 4, 7, 6)
                for pair in range(4):
                    emit_qkt("q", pair, 0)
                    emit_qkt("k", pair, 0)
                    if pair >= 1:
                        emit_h0_head(H0[pair - 1])
                for k, i in enumerate(range(4, 8)):
                    emit_v(i)
                    emit_h0_head(H0[3 + k])
                emit_h0_head(H0[7])
                for pair in range(4):
                    emit_qkt("q", pair, 512)
                    emit_qkt("k", pair, 512)

            # ---- half-1 attention + output projection ----
            with (
                tc.tile_pool(name="lg1", bufs=2, space="PSUM") as lg1p,
                tc.tile_pool(name="ap1", bufs=2, space="PSUM") as ap1p,
                tc.tile_pool(name="ypt", bufs=2, space="PSUM") as yptp,
                tc.tile_pool(name="pt1", bufs=3) as pt1p,
            ):
                def emit_h1_head(h):
                    # half-1: q in [512,1024), k-tiles 0..7, exp merged per
                    # j-pair with packed columns
                    pair, poff = h // 2, 64 * (h % 2)
                    ap = ap1p.tile([P, 512], F32, tag="ap", name=f"ap1_{h}")
                    pend = []
                    for jp in range(4):
                        js = []
                        off = 0
                        lg = lg1p.tile([P, 1024], F32, tag="lg", name="lg1")
                        pt = pt1p.tile([P, 1024], BF16, tag="pt", name="pt1")
                        for j in (2 * jp, 2 * jp + 1):
                            lo = max(512, 128 * j)
                            n = 1024 - lo
                            nc.tensor.matmul(
                                lg[:, off : off + n],
                                lhsT=kt_sb[pair][poff : poff + DK, ts(j, P)],
                                rhs=qt_sb[pair][poff : poff + DK, lo:1024],
                                start=True, stop=True,
                            )
                            js.append((j, lo, off, n))
                            off += n
                        nc.scalar.activation(
                            out=pt[:, 0:off], in_=lg[:, 0:off], func=EXP,
                            scale=0.125,
                        )
                        for j, lo, o, n in js:
                            if 128 * j >= 512:  # diagonal block in this half
                                nc.gpsimd.tensor_mul(
                                    pt[:, o : o + P], pt[:, o : o + P],
                                    ztril[:, :],
                                )
                        pend.append((jp, js, pt))
                        while len(pend) > 1:
                            ev1(h, ap, *pend.pop(0))
                    for args in pend:
                        ev1(h, ap, *args)
                    normalize(h, ap, 512)

                def ev1(h, ap, jp, js, pt):
                    for j, lo, off, n in js:
                        nc.tensor.matmul(
                            ap[0 : DK + 1, lo - 512 : 512],
                            lhsT=v_sb[j][:, h, :], rhs=pt[:, off : off + n],
                            start=(j == 0), stop=(j == NS - 1),
                            skip_group_check=True,
                        )

                def emit_outproj(o, hx, copy_eng):
                    ypt = yptp.tile([P, 512], F32, tag="y", name=f"y{o}_{hx}")
                    for c in range(DG // P):
                        nc.tensor.matmul(
                            ypt[:, :],
                            lhsT=wo_all[:, c, ts(o, P)],
                            rhs=at_sb[c][:, hx : hx + 512],
                            start=(c == 0), stop=(c == DG // P - 1),
                        )
                    ysb = ysp.tile([P, 512], F32, tag="ysb", name="ysb")
                    if copy_eng == "act":
                        nc.scalar.copy(out=ysb[:, :], in_=ypt[:, :])
                    else:
                        nc.vector.tensor_copy(ysb[:, :], ypt[:, :])
                    nc.sync.dma_start(out=yT[ts(o, P), hx : hx + 512], in_=ysb[:, :])

                H1 = (1, 0, 3, 2, 5, 4, 7, 6)
                for k, h in enumerate(H1):
                    emit_h1_head(h)
                    emit_outproj(k, 0, "dve")
                for o in range(ND):
                    emit_outproj(o, 512, "act")

    _split_excess_waits(nc)
    return nc


_NC_CACHE = {}


def _get_nc():
    if "nc" not in _NC_CACHE:
        _NC_CACHE["nc"] = build_nc()
    return _NC_CACHE["nc"]


# rotate-half permutation within each head: evens then odds
_PERM = np.concatenate([np.arange(0, DK, 2), np.arange(1, DK, 2)])


def _bf16(a):
    import ml_dtypes

    return np.asarray(a, dtype=ml_dtypes.bfloat16)


def _host_prep(x, Wq, Wk, Wv, Wo, token_positions):
    """Build the 8 per-core input dicts."""
    inv_freq = 1.0 / (ROPE_THETA ** (np.arange(0, DK, 2, dtype=np.float32) / DK))
    in_maps = []
    for core in range(8):
        b, g = core // 2, core % 2
        heads = np.arange(HG * g, HG * (g + 1))
        rows_qk = (heads[:, None] * DK + _PERM[None, :]).reshape(-1)
        rows_v = (heads[:, None] * DK + np.arange(DK)[None, :]).reshape(-1)
        pos = token_positions[b].astype(np.float32)  # [S]
        ang = pos[None, :] * inv_freq[:, None]  # [32, S]
        cosT = np.tile(np.cos(ang), (4, 1)).astype(np.float32)  # [128, S]
        sin = np.sin(ang)
        sinTs = np.concatenate([-sin, sin, -sin, sin], axis=0).astype(np.float32)
        in_maps.append(
            {
                "xT": _bf16(x[b].T),
                "wqT": _bf16(Wq[rows_qk, :].T),
                "wkT": _bf16(Wk[rows_qk, :].T),
                "wvT": _bf16(Wv[rows_v, :].T),
                "woT": _bf16(Wo[:, rows_v].T),
                "cosT": _bf16(cosT),
                "sinTs": _bf16(sinTs),
            }
        )
    return in_maps


def kernel(x, Wq, Wk, Wv, Wo, token_positions, _trace=False):
    x = np.asarray(x, dtype=np.float32)
    Wq = np.asarray(Wq, dtype=np.float32)
    Wk = np.asarray(Wk, dtype=np.float32)
    Wv = np.asarray(Wv, dtype=np.float32)
    Wo = np.asarray(Wo, dtype=np.float32)
    token_positions = np.asarray(token_positions)

    nc = _get_nc()
    in_maps = _host_prep(x, Wq, Wk, Wv, Wo, token_positions)
    res = run_bass_kernel_spmd(nc, in_maps, core_ids=list(range(8)), trace=_trace)
    if _trace:
        kernel.last_exec_time_ns = res.exec_time_ns
        kernel.last_results = res

    y = np.empty((B, S, D), dtype=np.float32)
    for b in range(B):
        yT0 = np.asarray(res.results[2 * b]["yT"], dtype=np.float32)
        yT1 = np.asarray(res.results[2 * b + 1]["yT"], dtype=np.float32)
        y[b] = (yT0 + yT1).T
    return y


# revision 26
# speedup vs baseline: 1.3337x; 1.0456x over previous
"""Multi-head self-attention (B=4, S=1024, D=1024, H=16, RoPE, causal) on 8
Trainium2 NeuronCores.

Sharding: 8 cores = 4 batches x 2 head-groups (8 heads each). Each core
computes QKV projections for its batch/head-group, RoPE, causal attention,
and a partial output projection (contraction over its 512 attention dims).
The host sums the two partial outputs per batch (the "all-reduce") and
concatenates batches.

v2 design (vs the f32r baseline):
- All matmul operands are bf16 (1 cyc/row at any tile size; half the DMA).
- Q/K are projected TRANSPOSED (weights stationary, x streamed) so no PE
  transposes are needed; RoPE runs in [dg, s] layout using a partition
  block-swap done by 4 small SBUF->SBUF DMAs plus 3 elementwise ops
  (2 on DVE, the add on GpSimd).
- Causal attention is split into two q-halves. Half-0 (q in [0,512), k-tiles
  0..3) only needs the first half of phase A, so its 8 heads interleave with
  the second half of the projections, overlapping ACT exp time with PE time.
- Logits are computed transposed (L^T[k, q]); softmax sums come from a ones
  column appended to V; exp instructions are merged across j k-tiles
  (contiguous packed columns) to amortize ACT overhead.
- Softmax 1/sum: DVE reciprocal_approx_fast on the [1,512] sum row, then a
  K=1 PE outer-product broadcasts it across 64 partitions (no DRAM round
  trip), then one DVE multiply produces bf16 normalized attention.
- The causal tril mask multiply runs on GpSimd (otherwise idle).
- Output projection per q-half interleaves with the other half's attention.
"""

import numpy as np

import concourse.bass as bass
import concourse.mybir as mybir
import concourse.tile as tile
from concourse.bass import ts
from concourse.bass_utils import run_bass_kernel_spmd
from concourse.masks import make_upper_triangular

B, S, D = 4, 1024, 1024
H = 16  # total heads
HG = 8  # heads per core (head-group)
DK = 64  # head dim
DG = HG * DK  # 512, per-core projection width
ROPE_THETA = 10000.0
P = 128  # partitions
NS = S // P  # 8 s-tiles
ND = D // P  # 8 d-chunks
F32 = mybir.dt.float32
F32R = mybir.dt.float32r
BF16 = mybir.dt.bfloat16
EXP = mybir.ActivationFunctionType.Exp

_uid = [0]


def _split_excess_waits(nc, limit=1):
    """This container's walrus rejects >1 sync waits on the kernel-tail
    Drain; move excess waits onto same-engine NoOps inserted before it."""
    for f in nc.m.functions:
        for blk in f.blocks:
            insts = list(blk.instructions)
            out = []
            changed = False
            for inst in insts:
                si = inst.sync_info
                if si is not None and si.on_wait and len(si.on_wait) > limit:
                    waits = list(si.on_wait)
                    head, tail = waits[:-limit], waits[-limit:]
                    for i in range(0, len(head), limit):
                        _uid[0] += 1
                        nop = mybir.InstNoOp(
                            name=f"waitsplit-{_uid[0]}", ins=[], outs=[]
                        )
                        nop.engine = inst.engine
                        nop.sync_info = mybir.SyncInfo(
                            on_wait=head[i : i + limit], on_update=[]
                        )
                        out.append(nop)
                    si.on_wait = tail
                    changed = True
                out.append(inst)
            if changed:
                blk.instructions = out
    return nc


def build_nc():
    nc = bass.Bass("TRN2")
    xT = nc.dram_tensor("xT", [D, S], BF16, kind="ExternalInput")
    wqT = nc.dram_tensor("wqT", [D, DG], BF16, kind="ExternalInput")
    wkT = nc.dram_tensor("wkT", [D, DG], BF16, kind="ExternalInput")
    wvT = nc.dram_tensor("wvT", [D, DG], BF16, kind="ExternalInput")
    woT = nc.dram_tensor("woT", [DG, D], BF16, kind="ExternalInput")
    cosT = nc.dram_tensor("cosT", [P, S], BF16, kind="ExternalInput")
    sinTs = nc.dram_tensor("sinTs", [P, S], BF16, kind="ExternalInput")
    pswT = nc.dram_tensor("pswT", [P, P], BF16, kind="ExternalInput")
    yT = nc.dram_tensor("yT", [D, S], BF16, kind="ExternalOutput")
    # DRAM scratch rows for the softmax 1/sum reshape + partition-broadcast
    rsum = nc.dram_tensor("rsum", [H, 512], F32)
    rbcd = nc.dram_tensor("rbcd", [H, 512], F32)

    with tile.TileContext(nc) as tc:
        with (
            tc.tile_pool(name="const", bufs=1) as constp,
            tc.tile_pool(name="wts", bufs=1) as wp,
            tc.tile_pool(name="big", bufs=1) as bigp,
            tc.tile_pool(name="qsw", bufs=3) as qswp,
            tc.tile_pool(name="rr", bufs=2) as rrp,
            tc.tile_pool(name="odd", bufs=2) as oddp,
            tc.tile_pool(name="ysb", bufs=3) as ysp,
        ):
            # ---- constants ----
            ztrilf = constp.tile([P, P], F32, tag="ztrilf")
            nc.vector.memset(ztrilf[:, :], 0.0)
            make_upper_triangular(nc, ztrilf[:, :], val=1.0, diag=True)
            ztril = constp.tile([P, P], BF16, tag="ztril")
            nc.vector.tensor_copy(ztril[:, :], ztrilf[:, :])

            # ---- resident weights/activations ----
            xs = wp.tile([P, ND, S], BF16, tag="xs", name="xs")
            wq_all = wp.tile([P, ND, DG], BF16, tag="wq", name="wq")
            wk_all = wp.tile([P, ND, DG], BF16, tag="wk", name="wk")
            wv_all = wp.tile([P, ND, DG], BF16, tag="wv", name="wv")
            wo_all = wp.tile([P, DG // P, D], BF16, tag="wo", name="wo")
            cs = wp.tile([P, S], BF16, tag="cs", name="cs")
            sn = wp.tile([P, S], BF16, tag="sn", name="sn")
            psw = wp.tile([P, P], BF16, tag="psw", name="psw")

            # xs per-chunk on sync queue (first V matmuls unblock early);
            # weights on the ACT hwdge queue run concurrently.
            for c in range(ND):
                nc.sync.dma_start(
                    out=xs[:, c, :], in_=xT[ts(c, P), :]
                )
            nc.scalar.dma_start(
                out=wv_all[:, :, :],
                in_=wvT[:, :].rearrange("(c p) o -> p c o", p=P),
            )
            nc.scalar.dma_start(out=cs[:, :], in_=cosT[:, :])
            nc.scalar.dma_start(out=sn[:, :], in_=sinTs[:, :])
            nc.scalar.dma_start(out=psw[:, :], in_=pswT[:, :])
            nc.scalar.dma_start(
                out=wq_all[:, :, :],
                in_=wqT[:, :].rearrange("(c p) o -> p c o", p=P),
            )
            nc.sync.dma_start(
                out=wk_all[:, :, :],
                in_=wkT[:, :].rearrange("(c p) o -> p c o", p=P),
            )
            nc.scalar.dma_start(
                out=wo_all[:, :, :],
                in_=woT[:, :].rearrange("(c p) o -> p c o", p=P),
            )

            # persistent: q^T/k^T pair tiles [128 dims, S], v tiles, at tiles
            qt_sb = [bigp.tile([P, S], BF16, tag=f"qt{p}", name=f"qt{p}") for p in range(4)]
            kt_sb = [bigp.tile([P, S], BF16, tag=f"kt{p}", name=f"kt{p}") for p in range(4)]
            v_sb = [bigp.tile([P, HG, DK + 1], BF16, tag=f"v{j}", name=f"v{j}") for j in range(NS)]
            at_sb = [bigp.tile([P, S], BF16, tag=f"at{p}", name=f"at{p}") for p in range(4)]
            for j in range(NS):
                nc.vector.memset(v_sb[j][:, :, DK : DK + 1], 1.0)

            with (
                tc.tile_pool(name="projv", bufs=3, space="PSUM") as pvp,
                tc.tile_pool(name="lg0", bufs=3, space="PSUM") as lg0p,
                tc.tile_pool(name="ap0", bufs=2, space="PSUM") as ap0p,
                tc.tile_pool(name="pt0", bufs=5) as pt0p,
            ):
                def emit_v(i):
                    vp = pvp.tile([P, DG], F32, tag="pv", name=f"v{i}")
                    for c in range(ND):
                        nc.tensor.matmul(
                            vp[:, :], lhsT=xs[:, c, ts(i, P)], rhs=wv_all[:, c, :],
                            start=(c == 0), stop=(c == ND - 1),
                        )
                    nc.scalar.copy(
                        out=v_sb[i][:, :, 0:DK],
                        in_=vp[:, :].rearrange("p (h c) -> p h c", h=HG),
                    )

                def emit_qkt(src, pair, hx):
                    # transposed projection of q or k pair tile, cols [hx, hx+512)
                    w_all = wq_all if src == "q" else wk_all
                    dst = qt_sb[pair] if src == "q" else kt_sb[pair]
                    pp = pvp.tile([P, DG], F32, tag="pv", name=f"{src}{pair}_{hx}")
                    for c in range(ND):
                        nc.tensor.matmul(
                            pp[:, :], lhsT=w_all[:, c, ts(pair, P)],
                            rhs=xs[:, c, hx : hx + 512],
                            start=(c == 0), stop=(c == ND - 1),
                        )
                    # rope: dst = pp*cos + blockswap(pp)*sin_signed; the
                    # block swap is a PE permutation matmul (psw)
                    qs = qswp.tile([P, 512], BF16, tag="qs", name="qs")
                    nc.vector.tensor_copy(qs[:, :], pp[:, :])
                    qw = pvp.tile([P, DG], F32, tag="pv", name="qw")
                    nc.tensor.matmul(
                        qw[:, :], lhsT=psw[:, :], rhs=qs[:, :],
                        start=True, stop=True,
                    )
                    t1 = qswp.tile([P, 512], BF16, tag="t1", name="t1")
                    nc.vector.tensor_mul(t1[:, :], qs[:, :], cs[:, hx : hx + 512])
                    t2 = qswp.tile([P, 512], BF16, tag="t2", name="t2")
                    nc.vector.tensor_mul(t2[:, :], qw[:, :], sn[:, hx : hx + 512])
                    nc.vector.tensor_add(dst[:, hx : hx + 512], t1[:, :], t2[:, :])

                def normalize(h, ap, hx):
                    # softmax 1/sum: copy s-row to SBUF, reshape to [128,4]
                    # (fast reciprocal shape) via SBUF DMA, reciprocal, DRAM
                    # round trip to broadcast across 64 partitions, one DVE
                    # multiply -> bf16 at tile
                    pair, poff = h // 2, 64 * (h % 2)
                    sr = rrp.tile([1, 512], F32, tag="rr", name="sr")
                    if hx == 0:
                        nc.scalar.copy(out=sr[:, :], in_=ap[DK : DK + 1, 0:512])
                    else:
                        nc.vector.tensor_copy(sr[:, :], ap[DK : DK + 1, 0:512])
                    slot = h + (HG if hx else 0)
                    nc.sync.dma_start(
                        out=rsum[slot, :].rearrange("(o c) -> o c", o=1),
                        in_=sr[:, :],
                    )
                    rs = rrp.tile([P, 4], F32, tag="rs", name="rs")
                    nc.sync.dma_start(
                        out=rs[:, :],
                        in_=rsum[slot, :].rearrange("(p c) -> p c", c=4),
                    )
                    rc = rrp.tile([P, 4], F32, tag="rc", name="rc")
                    nc.vector.reciprocal(out=rc[:, :], in_=rs[:, :])
                    nc.sync.dma_start(
                        out=rbcd[slot, :].rearrange("(p c) -> p c", c=4),
                        in_=rc[:, :],
                    )
                    row = rbcd[slot, :]
                    bc_src = bass.AP(
                        tensor=row.tensor,
                        offset=row.offset,
                        ap=[[0, DK], [1, 512]],
                    )
                    rbc = rrp.tile([DK, 512], F32, tag="rbc", name="rbc")
                    nc.sync.dma_start(out=rbc[:, :], in_=bc_src)
                    if poff == 0:
                        nc.vector.tensor_mul(
                            at_sb[pair][0:DK, hx : hx + 512],
                            ap[0:DK, 0:512], rbc[:, :],
                        )
                    else:
                        tmp = oddp.tile([DK, 512], BF16, tag="odd", name="odd")
                        nc.vector.tensor_mul(
                            tmp[:, :], ap[0:DK, 0:512], rbc[:, :]
                        )
                        nc.sync.dma_start(
                            out=at_sb[pair][DK:P, hx : hx + 512], in_=tmp[:, :]
                        )

                def emit_h0_head(h, extra=None):
                    # half-0: q in [0,512), k-tiles 0..3
                    pair, poff = h // 2, 64 * (h % 2)
                    ap = ap0p.tile([P, 512], F32, tag="ap", name=f"ap0_{h}")
                    pend = []
                    for j in range(4):
                        q0 = 128 * j
                        n = 512 - q0
                        lg = lg0p.tile([P, 512], F32, tag="lg", name="lg0")
                        nc.tensor.matmul(
                            lg[:, 0:n],
                            lhsT=kt_sb[pair][poff : poff + DK, ts(j, P)],
                            rhs=qt_sb[pair][poff : poff + DK, q0:512],
                            start=True, stop=True,
                        )
                        pt = pt0p.tile([P, 512], BF16, tag="pt", name="pt0")
                        nc.scalar.activation(
                            out=pt[:, 0:n], in_=lg[:, 0:n], func=EXP, scale=0.125
                        )
                        nc.gpsimd.tensor_mul(
                            pt[:, 0:P], pt[:, 0:P], ztril[:, :]
                        )
                        pend.append((j, q0, n, pt))
                        while len(pend) > 2:
                            ev0(h, ap, *pend.pop(0))
                    if extra is not None:
                        extra()  # PE filler while the tail exp/tril drain
                    for args in pend:
                        ev0(h, ap, *args)
                    normalize(h, ap, 0)

                def ev0(h, ap, j, q0, n, pt):
                    nc.tensor.matmul(
                        ap[0 : DK + 1, q0:512],
                        lhsT=v_sb[j][:, h, :], rhs=pt[:, 0:n],
                        start=(j == 0), stop=(j == 3),
                        skip_group_check=True,
                    )

                # ---- emission: phase A half-0, half-0 attn, phase A half-1
                for i in range(4):
                    emit_v(i)
                H0 = (1, 0, 3, 2, 5, 4, 7, 6)
                for pair in range(4):
                    emit_qkt("q", pair, 0)
                    emit_qkt("k", pair, 0)
                    if pair >= 1:
                        emit_h0_head(H0[pair - 1])
                for k, i in enumerate(range(4, 8)):
                    emit_v(i)
                    if k == 0:
                        emit_h0_head(H0[3])
                    else:
                        pr = k - 1
                        emit_h0_head(
                            H0[3 + k],
                            extra=lambda pr=pr: (
                                emit_qkt("q", pr, 512),
                                emit_qkt("k", pr, 512),
                            ),
                        )
                emit_h0_head(
                    H0[7],
                    extra=lambda: (
                        emit_qkt("q", 3, 512),
                        emit_qkt("k", 3, 512),
                    ),
                )

            # ---- half-1 attention + output projection ----
            with (
                tc.tile_pool(name="lg1", bufs=2, space="PSUM") as lg1p,
                tc.tile_pool(name="ap1", bufs=2, space="PSUM") as ap1p,
                tc.tile_pool(name="ypt", bufs=2, space="PSUM") as yptp,
                tc.tile_pool(name="pt1", bufs=4) as pt1p,
            ):
                def h1_qk(h, ap, jp):
                    # one j-pair of half-1 QK + merged exp + tril for head h
                    pair, poff = h // 2, 64 * (h % 2)
                    js = []
                    off = 0
                    lg = lg1p.tile([P, 1024], F32, tag="lg", name="lg1")
                    pt = pt1p.tile([P, 1024], BF16, tag="pt", name="pt1")
                    for j in (2 * jp, 2 * jp + 1):
                        lo = max(512, 128 * j)
                        n = 1024 - lo
                        nc.tensor.matmul(
                            lg[:, off : off + n],
                            lhsT=kt_sb[pair][poff : poff + DK, ts(j, P)],
                            rhs=qt_sb[pair][poff : poff + DK, lo:1024],
                            start=True, stop=True,
                        )
                        js.append((j, lo, off, n))
                        off += n
                    nc.scalar.activation(
                        out=pt[:, 0:off], in_=lg[:, 0:off], func=EXP,
                        scale=0.125,
                    )
                    for j, lo, o, n in js:
                        if 128 * j >= 512:  # diagonal block in this half
                            nc.gpsimd.tensor_mul(
                                pt[:, o : o + P], pt[:, o : o + P],
                                ztril[:, :],
                            )
                    return (js, pt)

                def ev1(h, ap, js, pt):
                    for j, lo, off, n in js:
                        nc.tensor.matmul(
                            ap[0 : DK + 1, lo - 512 : 512],
                            lhsT=v_sb[j][:, h, :], rhs=pt[:, off : off + n],
                            start=(j == 0), stop=(j == NS - 1),
                            skip_group_check=True,
                        )

                def emit_h1_pair(ha, hb, extra=None):
                    # two heads of one pair interleaved: doubles independent
                    # work in flight so exp latency hides under the other
                    # head's matmuls
                    apa = ap1p.tile([P, 512], F32, tag="ap", name=f"ap1_{ha}")
                    apb = ap1p.tile([P, 512], F32, tag="ap", name=f"ap1_{hb}")
                    pend = []
                    for jp in range(4):
                        pend.append((ha, apa, h1_qk(ha, apa, jp)))
                        pend.append((hb, apb, h1_qk(hb, apb, jp)))
                        while len(pend) > 2:
                            h, ap, (js, pt) = pend.pop(0)
                            ev1(h, ap, js, pt)
                    if extra is not None:
                        extra()
                    for h, ap, (js, pt) in pend:
                        ev1(h, ap, js, pt)
                    normalize(ha, apa, 512)
                    normalize(hb, apb, 512)

                def emit_outproj(o, hx, copy_eng):
                    ypt = yptp.tile([P, 512], F32, tag="y", name=f"y{o}_{hx}")
                    for c in range(DG // P):
                        nc.tensor.matmul(
                            ypt[:, :],
                            lhsT=wo_all[:, c, ts(o, P)],
                            rhs=at_sb[c][:, hx : hx + 512],
                            start=(c == 0), stop=(c == DG // P - 1),
                        )
                    ysb = ysp.tile([P, 512], BF16, tag="ysb", name="ysb")
                    if copy_eng == "act":
                        nc.scalar.copy(out=ysb[:, :], in_=ypt[:, :])
                    else:
                        nc.vector.tensor_copy(ysb[:, :], ypt[:, :])
                    nc.sync.dma_start(out=yT[ts(o, P), hx : hx + 512], in_=ysb[:, :])

                for pr in range(4):
                    emit_h1_pair(
                        2 * pr + 1, 2 * pr,
                        extra=lambda pr=pr: (
                            emit_outproj(2 * pr, 0, "dve"),
                            emit_outproj(2 * pr + 1, 0, "dve"),
                        ),
                    )
                for o in range(ND):
                    emit_outproj(o, 512, "act")

    _split_excess_waits(nc)
    return nc


_NC_CACHE = {}


def _get_nc():
    if "nc" not in _NC_CACHE:
        _NC_CACHE["nc"] = build_nc()
    return _NC_CACHE["nc"]


# rotate-half permutation within each head: evens then odds
_PERM = np.concatenate([np.arange(0, DK, 2), np.arange(1, DK, 2)])


def _bf16(a):
    import ml_dtypes

    return np.asarray(a, dtype=ml_dtypes.bfloat16)


def _host_prep(x, Wq, Wk, Wv, Wo, token_positions):
    """Build the 8 per-core input dicts."""
    inv_freq = 1.0 / (ROPE_THETA ** (np.arange(0, DK, 2, dtype=np.float32) / DK))
    in_maps = []
    for core in range(8):
        b, g = core // 2, core % 2
        heads = np.arange(HG * g, HG * (g + 1))
        rows_qk = (heads[:, None] * DK + _PERM[None, :]).reshape(-1)
        rows_v = (heads[:, None] * DK + np.arange(DK)[None, :]).reshape(-1)
        pos = token_positions[b].astype(np.float32)  # [S]
        ang = pos[None, :] * inv_freq[:, None]  # [32, S]
        cosT = np.tile(np.cos(ang), (4, 1)).astype(np.float32)  # [128, S]
        sin = np.sin(ang)
        sinTs = np.concatenate([-sin, sin, -sin, sin], axis=0).astype(np.float32)
        psw = np.zeros((P, P), dtype=np.float32)
        psw[np.arange(P) ^ 32, np.arange(P)] = 1.0
        in_maps.append(
            {
                "xT": _bf16(x[b].T),
                "wqT": _bf16(Wq[rows_qk, :].T),
                "wkT": _bf16(Wk[rows_qk, :].T),
                "wvT": _bf16(Wv[rows_v, :].T),
                "woT": _bf16(Wo[:, rows_v].T),
                "cosT": _bf16(cosT),
                "sinTs": _bf16(sinTs),
                "pswT": _bf16(psw),
            }
        )
    return in_maps


def kernel(x, Wq, Wk, Wv, Wo, token_positions, _trace=False):
    x = np.asarray(x, dtype=np.float32)
    Wq = np.asarray(Wq, dtype=np.float32)
    Wk = np.asarray(Wk, dtype=np.float32)
    Wv = np.asarray(Wv, dtype=np.float32)
    Wo = np.asarray(Wo, dtype=np.float32)
    token_positions = np.asarray(token_positions)

    nc = _get_nc()
    in_maps = _host_prep(x, Wq, Wk, Wv, Wo, token_positions)
    res = run_bass_kernel_spmd(nc, in_maps, core_ids=list(range(8)), trace=_trace)
    if _trace:
        kernel.last_exec_time_ns = res.exec_time_ns
        kernel.last_results = res

    y = np.empty((B, S, D), dtype=np.float32)
    for b in range(B):
        yT0 = np.asarray(res.results[2 * b]["yT"], dtype=np.float32)
        yT1 = np.asarray(res.results[2 * b + 1]["yT"], dtype=np.float32)
        y[b] = (yT0 + yT1).T
    return y


# revision 28
# speedup vs baseline: 1.3588x; 1.0188x over previous
"""Multi-head self-attention (B=4, S=1024, D=1024, H=16, RoPE, causal) on 8
Trainium2 NeuronCores.

Sharding: 8 cores = 4 batches x 2 head-groups (8 heads each). Each core
computes QKV projections for its batch/head-group, RoPE, causal attention,
and a partial output projection (contraction over its 512 attention dims).
The host sums the two partial outputs per batch (the "all-reduce") and
concatenates batches.

Design (v4):
- All matmul operands bf16 (1 cyc/row at any tile size; half the DMA).
- Q/K are projected TRANSPOSED (weights stationary, x streamed): no PE
  transposes. RoPE runs in [dg, s] layout; the even/odd partner rows come
  from a PE permutation matmul (psw), then 3 DVE elementwise ops.
- Causal attention splits into two q-halves. Half-0 (q<512, k-tiles 0..3)
  interleaves with the second half of the projections so ACT exp time hides
  under PE projection time. The transposed-logits layout (L^T[k,q]) gives
  softmax sums via a ones column in V.
- Softmax 1/sum per head pair: two s-rows batched through one reshape DMA,
  one [128,8] reciprocal, one broadcast DMA; normalization multiplies write
  even heads to partitions 0:64 and odd heads directly to 64:128
  (cross-partition-offset engine writes work at 32-partition granularity).
- tril masking on GpSimd (idle otherwise); exp on ACT; everything else
  balanced between DVE/ACT per phase.
- Output projection per q-half interleaves with the other half's attention;
  y is written bf16 (host sums the two partial outputs in f32).
"""

import numpy as np

import concourse.bass as bass
import concourse.mybir as mybir
import concourse.tile as tile
from concourse.bass import ts
from concourse.bass_utils import run_bass_kernel_spmd
from concourse.masks import make_upper_triangular

B, S, D = 4, 1024, 1024
H = 16  # total heads
HG = 8  # heads per core (head-group)
DK = 64  # head dim
DG = HG * DK  # 512, per-core projection width
ROPE_THETA = 10000.0
P = 128  # partitions
NS = S // P  # 8 s-tiles
ND = D // P  # 8 d-chunks
F32 = mybir.dt.float32
BF16 = mybir.dt.bfloat16
EXP = mybir.ActivationFunctionType.Exp

_uid = [0]


def _split_excess_waits(nc, limit=1):
    """This container's walrus rejects >1 sync waits on the kernel-tail
    Drain; move excess waits onto same-engine NoOps inserted before it."""
    for f in nc.m.functions:
        for blk in f.blocks:
            insts = list(blk.instructions)
            out = []
            changed = False
            for inst in insts:
                si = inst.sync_info
                if si is not None and si.on_wait and len(si.on_wait) > limit:
                    waits = list(si.on_wait)
                    head, tail = waits[:-limit], waits[-limit:]
                    for i in range(0, len(head), limit):
                        _uid[0] += 1
                        nop = mybir.InstNoOp(
                            name=f"waitsplit-{_uid[0]}", ins=[], outs=[]
                        )
                        nop.engine = inst.engine
                        nop.sync_info = mybir.SyncInfo(
                            on_wait=head[i : i + limit], on_update=[]
                        )
                        out.append(nop)
                    si.on_wait = tail
                    changed = True
                out.append(inst)
            if changed:
                blk.instructions = out
    return nc


def build_nc():
    nc = bass.Bass("TRN2")
    xT = nc.dram_tensor("xT", [D, S], BF16, kind="ExternalInput")
    wqT = nc.dram_tensor("wqT", [D, DG], BF16, kind="ExternalInput")
    wkT = nc.dram_tensor("wkT", [D, DG], BF16, kind="ExternalInput")
    wvT = nc.dram_tensor("wvT", [D, DG], BF16, kind="ExternalInput")
    woT = nc.dram_tensor("woT", [DG, D], BF16, kind="ExternalInput")
    cosT = nc.dram_tensor("cosT", [P, S], BF16, kind="ExternalInput")
    sinTs = nc.dram_tensor("sinTs", [P, S], BF16, kind="ExternalInput")
    pswT = nc.dram_tensor("pswT", [P, P], BF16, kind="ExternalInput")
    yT = nc.dram_tensor("yT", [D, S], BF16, kind="ExternalOutput")
    # DRAM scratch for the softmax 1/sum reshape + partition-broadcast
    # (one 1024-wide slot per head-pair-half: [0:512]=odd head, [512:]=even)
    rsum = nc.dram_tensor("rsum", [HG, 2 * 512], F32)
    rbcd = nc.dram_tensor("rbcd", [HG, 2 * 512], F32)

    with tile.TileContext(nc) as tc:
        with (
            tc.tile_pool(name="const", bufs=1) as constp,
            tc.tile_pool(name="wts", bufs=1) as wp,
            tc.tile_pool(name="big", bufs=1) as bigp,
            tc.tile_pool(name="qsw", bufs=3) as qswp,
            tc.tile_pool(name="rr", bufs=2) as rrp,
            tc.tile_pool(name="ysb", bufs=3) as ysp,
        ):
            # ---- constants ----
            ztrilf = constp.tile([P, P], F32, tag="ztrilf")
            nc.vector.memset(ztrilf[:, :], 0.0)
            make_upper_triangular(nc, ztrilf[:, :], val=1.0, diag=True)
            ztril = constp.tile([P, P], BF16, tag="ztril")
            nc.vector.tensor_copy(ztril[:, :], ztrilf[:, :])

            # ---- resident weights/activations ----
            xs = wp.tile([P, ND, S], BF16, tag="xs", name="xs")
            wq_all = wp.tile([P, ND, DG], BF16, tag="wq", name="wq")
            wk_all = wp.tile([P, ND, DG], BF16, tag="wk", name="wk")
            wv_all = wp.tile([P, ND, DG], BF16, tag="wv", name="wv")
            wo_all = wp.tile([P, DG // P, D], BF16, tag="wo", name="wo")
            cs = wp.tile([P, S], BF16, tag="cs", name="cs")
            sn = wp.tile([P, S], BF16, tag="sn", name="sn")
            psw = wp.tile([P, P], BF16, tag="psw", name="psw")

            # xs per-chunk on the sync queue (first V matmuls unblock
            # early); weights on the ACT hwdge queue run concurrently.
            for c in range(ND):
                nc.sync.dma_start(out=xs[:, c, :], in_=xT[ts(c, P), :])
            nc.scalar.dma_start(
                out=wv_all[:, :, :],
                in_=wvT[:, :].rearrange("(c p) o -> p c o", p=P),
            )
            nc.scalar.dma_start(out=psw[:, :], in_=pswT[:, :])
            nc.scalar.dma_start(out=cs[:, :], in_=cosT[:, :])
            nc.scalar.dma_start(out=sn[:, :], in_=sinTs[:, :])
            nc.scalar.dma_start(
                out=wq_all[:, :, :],
                in_=wqT[:, :].rearrange("(c p) o -> p c o", p=P),
            )
            nc.sync.dma_start(
                out=wk_all[:, :, :],
                in_=wkT[:, :].rearrange("(c p) o -> p c o", p=P),
            )
            nc.scalar.dma_start(
                out=wo_all[:, :, :],
                in_=woT[:, :].rearrange("(c p) o -> p c o", p=P),
            )

            # persistent: q^T/k^T pair tiles [128 dims, S], v tiles, at tiles
            qt_sb = [bigp.tile([P, S], BF16, tag=f"qt{p}", name=f"qt{p}") for p in range(4)]
            kt_sb = [bigp.tile([P, S], BF16, tag=f"kt{p}", name=f"kt{p}") for p in range(4)]
            v_sb = [bigp.tile([P, HG, DK + 1], BF16, tag=f"v{j}", name=f"v{j}") for j in range(NS)]
            at_sb = [bigp.tile([P, S], BF16, tag=f"at{p}", name=f"at{p}") for p in range(4)]
            for j in range(NS):
                nc.vector.memset(v_sb[j][:, :, DK : DK + 1], 1.0)

            def normalize_pair(ha, apa, hb, apb, hx, eng):
                # batched softmax 1/sum for a head pair: copy both s-rows
                # into one tile (partitions 0 and 32), reshape via DRAM to
                # [128,8], one reciprocal, broadcast back, two multiplies.
                # Odd heads write at partitions 64:128 directly.
                sr2 = rrp.tile([P, 512], F32, tag="sr", name="sr2")
                if eng == "act":
                    nc.scalar.copy(out=sr2[0:1, :], in_=apa[DK : DK + 1, 0:512])
                    nc.scalar.copy(out=sr2[32:33, :], in_=apb[DK : DK + 1, 0:512])
                else:
                    nc.vector.tensor_copy(sr2[0:1, :], apa[DK : DK + 1, 0:512])
                    nc.vector.tensor_copy(sr2[32:33, :], apb[DK : DK + 1, 0:512])
                slot = (ha // 2) + (4 if hx else 0)
                src = bass.AP(
                    tensor=sr2[:, :].tensor,
                    offset=sr2[:, :].offset,
                    ap=[[32 * 512, 2], [1, 512]],
                )
                nc.sync.dma_start(
                    out=rsum[slot, :].rearrange("(o c) -> o c", o=2), in_=src
                )
                rs = rrp.tile([P, 8], F32, tag="rs", name="rs")
                nc.sync.dma_start(
                    out=rs[:, :], in_=rsum[slot, :].rearrange("(p c) -> p c", c=8)
                )
                rc = rrp.tile([P, 8], F32, tag="rc", name="rc")
                nc.vector.reciprocal(out=rc[:, :], in_=rs[:, :])
                nc.sync.dma_start(
                    out=rbcd[slot, :].rearrange("(p c) -> p c", c=8), in_=rc[:, :]
                )
                row = rbcd[slot, :]
                bc_src = bass.AP(
                    tensor=row.tensor, offset=row.offset, ap=[[0, DK], [1, 1024]]
                )
                rbc2 = rrp.tile([DK, 1024], F32, tag="rbc", name="rbc2")
                nc.sync.dma_start(out=rbc2[:, :], in_=bc_src)
                for h, ap, col in ((ha, apa, 0), (hb, apb, 512)):
                    pair, poff = h // 2, 64 * (h % 2)
                    nc.vector.tensor_mul(
                        at_sb[pair][poff : poff + DK, hx : hx + 512],
                        ap[0:DK, 0:512],
                        rbc2[:, col : col + 512],
                    )

            with (
                tc.tile_pool(name="projv", bufs=3, space="PSUM") as pvp,
                tc.tile_pool(name="lg0", bufs=2, space="PSUM") as lg0p,
                tc.tile_pool(name="ap0", bufs=3, space="PSUM") as ap0p,
                tc.tile_pool(name="pt0", bufs=5) as pt0p,
            ):
                def emit_v(i):
                    vp = pvp.tile([P, DG], F32, tag="pv", name=f"v{i}")
                    for c in range(ND):
                        nc.tensor.matmul(
                            vp[:, :], lhsT=xs[:, c, ts(i, P)], rhs=wv_all[:, c, :],
                            start=(c == 0), stop=(c == ND - 1),
                        )
                    nc.vector.tensor_copy(
                        v_sb[i][:, :, 0:DK],
                        vp[:, :].rearrange("p (h c) -> p h c", h=HG),
                    )

                def emit_qkt(src, pair, hx):
                    # transposed projection of q or k pair tile, cols [hx, hx+512)
                    w_all = wq_all if src == "q" else wk_all
                    dst = qt_sb[pair] if src == "q" else kt_sb[pair]
                    pp = pvp.tile([P, DG], F32, tag="pv", name=f"{src}{pair}_{hx}")
                    for c in range(ND):
                        nc.tensor.matmul(
                            pp[:, :], lhsT=w_all[:, c, ts(pair, P)],
                            rhs=xs[:, c, hx : hx + 512],
                            start=(c == 0), stop=(c == ND - 1),
                        )
                    # rope: dst = pp*cos + blockswap(pp)*sin_signed; the
                    # block swap is a PE permutation matmul (psw)
                    qs = qswp.tile([P, 512], BF16, tag="qs", name="qs")
                    nc.vector.tensor_copy(qs[:, :], pp[:, :])
                    qw = pvp.tile([P, DG], F32, tag="pv", name="qw")
                    nc.tensor.matmul(
                        qw[:, :], lhsT=psw[:, :], rhs=qs[:, :],
                        start=True, stop=True,
                    )
                    t1 = qswp.tile([P, 512], BF16, tag="t1", name="t1")
                    nc.vector.tensor_mul(t1[:, :], qs[:, :], cs[:, hx : hx + 512])
                    t2 = qswp.tile([P, 512], BF16, tag="t2", name="t2")
                    nc.vector.tensor_mul(t2[:, :], qw[:, :], sn[:, hx : hx + 512])
                    nc.vector.tensor_add(dst[:, hx : hx + 512], t1[:, :], t2[:, :])

                def ev0(h, ap, j, q0, n, pt):
                    nc.tensor.matmul(
                        ap[0 : DK + 1, q0:512],
                        lhsT=v_sb[j][:, h, :], rhs=pt[:, 0:n],
                        start=(j == 0), stop=(j == 3),
                        skip_group_check=True,
                    )

                def emit_h0_head(h, extra=None):
                    # half-0: q in [0,512), k-tiles 0..3
                    pair, poff = h // 2, 64 * (h % 2)
                    ap = ap0p.tile([P, 512], F32, tag="ap", name=f"ap0_{h}")
                    pend = []
                    for j in range(4):
                        q0 = 128 * j
                        n = 512 - q0
                        lg = lg0p.tile([P, 512], F32, tag="lg", name="lg0")
                        nc.tensor.matmul(
                            lg[:, 0:n],
                            lhsT=kt_sb[pair][poff : poff + DK, ts(j, P)],
                            rhs=qt_sb[pair][poff : poff + DK, q0:512],
                            start=True, stop=True,
                        )
                        pt = pt0p.tile([P, 512], BF16, tag="pt", name="pt0")
                        nc.scalar.activation(
                            out=pt[:, 0:n], in_=lg[:, 0:n], func=EXP, scale=0.125
                        )
                        nc.gpsimd.tensor_mul(pt[:, 0:P], pt[:, 0:P], ztril[:, :])
                        pend.append((j, q0, n, pt))
                        while len(pend) > 2:
                            ev0(h, ap, *pend.pop(0))
                    if extra is not None:
                        extra()  # PE filler while the tail exp/tril drain
                    for args in pend:
                        ev0(h, ap, *args)
                    return ap

                # ---- emission: phase A half-0, half-0 attn, phase A half-1
                for i in range(4):
                    emit_v(i)
                aps0 = {}
                for pair in range(4):
                    emit_qkt("q", pair, 0)
                    emit_qkt("k", pair, 0)
                    if pair == 1:
                        aps0[1] = emit_h0_head(1)
                    elif pair == 2:
                        aps0[0] = emit_h0_head(0)
                        normalize_pair(1, aps0.pop(1), 0, aps0.pop(0), 0, "act")
                    elif pair == 3:
                        aps0[3] = emit_h0_head(3)
                emit_v(4)
                aps0[2] = emit_h0_head(2)
                normalize_pair(3, aps0.pop(3), 2, aps0.pop(2), 0, "act")
                emit_v(5)
                aps0[5] = emit_h0_head(5, extra=lambda: (
                    emit_qkt("q", 0, 512), emit_qkt("k", 0, 512)))
                emit_v(6)
                aps0[4] = emit_h0_head(4, extra=lambda: (
                    emit_qkt("q", 1, 512), emit_qkt("k", 1, 512)))
                normalize_pair(5, aps0.pop(5), 4, aps0.pop(4), 0, "act")
                emit_v(7)
                aps0[7] = emit_h0_head(7, extra=lambda: (
                    emit_qkt("q", 2, 512), emit_qkt("k", 2, 512)))
                aps0[6] = emit_h0_head(6, extra=lambda: (
                    emit_qkt("q", 3, 512), emit_qkt("k", 3, 512)))
                normalize_pair(7, aps0.pop(7), 6, aps0.pop(6), 0, "act")

            # ---- half-1 attention + output projection ----
            with (
                tc.tile_pool(name="lg1", bufs=3, space="PSUM") as lg1p,
                tc.tile_pool(name="ap1", bufs=3, space="PSUM") as ap1p,
                tc.tile_pool(name="ypt", bufs=2, space="PSUM") as yptp,
                tc.tile_pool(name="pt1", bufs=6) as pt1p,
            ):
                def h1_qk(h, j):
                    # one k-tile of half-1 QK + exp (+ tril for diag tiles)
                    pair, poff = h // 2, 64 * (h % 2)
                    lo = max(512, 128 * j)
                    n = 1024 - lo
                    lg = lg1p.tile([P, 512], F32, tag="lg", name="lg1")
                    nc.tensor.matmul(
                        lg[:, 0:n],
                        lhsT=kt_sb[pair][poff : poff + DK, ts(j, P)],
                        rhs=qt_sb[pair][poff : poff + DK, lo:1024],
                        start=True, stop=True,
                    )
                    pt = pt1p.tile([P, 512], BF16, tag="pt", name="pt1")
                    nc.scalar.activation(
                        out=pt[:, 0:n], in_=lg[:, 0:n], func=EXP, scale=0.125
                    )
                    if 128 * j >= 512:  # diagonal block leads this tile
                        nc.gpsimd.tensor_mul(pt[:, 0:P], pt[:, 0:P], ztril[:, :])
                    return (j, lo, n, pt)

                def ev1(h, ap, j, lo, n, pt):
                    nc.tensor.matmul(
                        ap[0 : DK + 1, lo - 512 : 512],
                        lhsT=v_sb[j][:, h, :], rhs=pt[:, 0:n],
                        start=(j == 0), stop=(j == NS - 1),
                        skip_group_check=True,
                    )

                def emit_h1_pair(ha, hb, extra=None):
                    # two heads of one pair interleaved per k-tile: doubles
                    # the independent work in flight
                    apa = ap1p.tile([P, 512], F32, tag="ap", name=f"ap1_{ha}")
                    apb = ap1p.tile([P, 512], F32, tag="ap", name=f"ap1_{hb}")
                    pend = []
                    for j in range(NS):
                        pend.append((ha, apa) + h1_qk(ha, j))
                        pend.append((hb, apb) + h1_qk(hb, j))
                        while len(pend) > 4:
                            a = pend.pop(0)
                            ev1(a[0], a[1], *a[2:])
                    if extra is not None:
                        extra()
                    for a in pend:
                        ev1(a[0], a[1], *a[2:])
                    normalize_pair(ha, apa, hb, apb, 512, "dve")

                def emit_outproj(o, hx, copy_eng):
                    ypt = yptp.tile([P, 512], F32, tag="y", name=f"y{o}_{hx}")
                    for c in range(DG // P):
                        nc.tensor.matmul(
                            ypt[:, :],
                            lhsT=wo_all[:, c, ts(o, P)],
                            rhs=at_sb[c][:, hx : hx + 512],
                            start=(c == 0), stop=(c == DG // P - 1),
                        )
                    ysb = ysp.tile([P, 512], BF16, tag="ysb", name="ysb")
                    if copy_eng == "act":
                        nc.scalar.copy(out=ysb[:, :], in_=ypt[:, :])
                    else:
                        nc.vector.tensor_copy(ysb[:, :], ypt[:, :])
                    nc.sync.dma_start(out=yT[ts(o, P), hx : hx + 512], in_=ysb[:, :])

                for pr in range(4):
                    emit_h1_pair(
                        2 * pr + 1, 2 * pr,
                        extra=lambda pr=pr: (
                            emit_outproj(2 * pr, 0, "dve"),
                            emit_outproj(2 * pr + 1, 0, "dve"),
                        ),
                    )
                for o in range(ND):
                    emit_outproj(o, 512, "act")

    _split_excess_waits(nc)
    return nc


_NC_CACHE = {}


def _get_nc():
    if "nc" not in _NC_CACHE:
        _NC_CACHE["nc"] = build_nc()
    return _NC_CACHE["nc"]


# rotate-half permutation within each head: evens then odds
_PERM = np.concatenate([np.arange(0, DK, 2), np.arange(1, DK, 2)])


def _bf16(a):
    import ml_dtypes

    return np.asarray(a, dtype=ml_dtypes.bfloat16)


def _host_prep(x, Wq, Wk, Wv, Wo, token_positions):
    """Build the 8 per-core input dicts."""
    inv_freq = 1.0 / (ROPE_THETA ** (np.arange(0, DK, 2, dtype=np.float32) / DK))
    in_maps = []
    for core in range(8):
        b, g = core // 2, core % 2
        heads = np.arange(HG * g, HG * (g + 1))
        rows_qk = (heads[:, None] * DK + _PERM[None, :]).reshape(-1)
        rows_v = (heads[:, None] * DK + np.arange(DK)[None, :]).reshape(-1)
        pos = token_positions[b].astype(np.float32)  # [S]
        ang = pos[None, :] * inv_freq[:, None]  # [32, S]
        cosT = np.tile(np.cos(ang), (4, 1)).astype(np.float32)  # [128, S]
        sin = np.sin(ang)
        sinTs = np.concatenate([-sin, sin, -sin, sin], axis=0).astype(np.float32)
        psw = np.zeros((P, P), dtype=np.float32)
        psw[np.arange(P) ^ 32, np.arange(P)] = 1.0
        in_maps.append(
            {
                "xT": _bf16(x[b].T),
                "wqT": _bf16(Wq[rows_qk, :].T),
                "wkT": _bf16(Wk[rows_qk, :].T),
                "wvT": _bf16(Wv[rows_v, :].T),
                "woT": _bf16(Wo[:, rows_v].T),
                "cosT": _bf16(cosT),
                "sinTs": _bf16(sinTs),
                "pswT": _bf16(psw),
            }
        )
    return in_maps


def kernel(x, Wq, Wk, Wv, Wo, token_positions, _trace=False):
    x = np.asarray(x, dtype=np.float32)
    Wq = np.asarray(Wq, dtype=np.float32)
    Wk = np.asarray(Wk, dtype=np.float32)
    Wv = np.asarray(Wv, dtype=np.float32)
    Wo = np.asarray(Wo, dtype=np.float32)
    token_positions = np.asarray(token_positions)

    nc = _get_nc()
    in_maps = _host_prep(x, Wq, Wk, Wv, Wo, token_positions)
    res = run_bass_kernel_spmd(nc, in_maps, core_ids=list(range(8)), trace=_trace)
    if _trace:
        kernel.last_exec_time_ns = res.exec_time_ns
        kernel.last_results = res

    y = np.empty((B, S, D), dtype=np.float32)
    for b in range(B):
        yT0 = np.asarray(res.results[2 * b]["yT"], dtype=np.float32)
        yT1 = np.asarray(res.results[2 * b + 1]["yT"], dtype=np.float32)
        y[b] = (yT0 + yT1).T
    return y
